# revision 5
# baseline (speedup 1.0000x reference)
"""Trainium2 Bass kernel: causal multi-head attention with softmax over the
QUERY axis (faithful to the reference's softmax(dim=-2) quirk).

Problem shapes: B=2, T=2048, E=1024, H=16, D=64.

Sharding: 8 cores = 2 batches x 4 head-groups (4 heads per core, zero
communication).  Host pre-transposes X to (E, T) per batch, packs per-head
weights into head-pair columns, and reassembles the output from per-core
(2, 128, 2048) tiles.

Per-core math (heads as two pairs (hA, hB)):
  Q_T, K_T: (d, t) layout via weight-stationary matmuls (2 heads packed into
            the 128-partition dim), bias added during PSUM->SBUF copy.
  V:        (t, d) layout via X_T-stationary matmuls (all 4 heads packed into
            the free dim), bias broadcast-added by DVE.
  S_T = K_T^T Q_T in (s, t) layout, computed only for t >= s0 (causal), so the
            softmax-over-queries is a free-axis reduction.  Triangular -1e30
            mask added on the 128-wide diagonal block only.
  exp on ACT with fused accum_out => denominators come free.
  Normalize V (2048x64 per head) by 1/denom instead of the O(T^2) weights.
  O_T accumulated in PSUM via V'-stationary matmuls (2 heads packed via PE
            column tiling), copied out and DMA'd as (d, t) rows.
"""

import numpy as np
from contextlib import ExitStack

B, T, E, H, D = 2, 2048, 1024, 16, 64
NCORES = 8
PAIRS = 2          # head pairs per core (4 heads)
EC = E // 128      # 8 contraction chunks
TB = T // 128      # 16 s-blocks
TC = T // 512      # 4 output column chunks
NEG = -1.0e30
SCALE = float(D) ** -0.5

_CACHE = {}


def _st_chunks(h0, h1):
    """Split [h0, h1) into <=512 pieces aligned to the psum tile's own 512
    grid (tile column 0 is at absolute t=h0)."""
    out = []
    c0 = h0
    while c0 < h1:
        w = min(512, h1 - c0)
        out.append((c0, w))
        c0 += w
    return out


def _av_chunks(s0):
    """Split [s0, 2048) on the absolute 512 grid (psum_o bank alignment)."""
    out = []
    for j in range(s0 // 512, TC):
        c0 = max(s0, 512 * j)
        out.append((j, c0, 512 * (j + 1) - c0))
    return out


def _emit(tc, io):
    """Emit the kernel program into TileContext tc.  io: dict name -> AP."""
    import concourse.bass as bass
    import concourse.mybir as mybir

    nc = tc.nc
    fp32 = mybir.dt.float32
    AF = mybir.ActivationFunctionType
    ALU = mybir.AluOpType

    x_t, wq, wk, wv = io["x_t"], io["wq"], io["wk"], io["wv"]
    bq, bk, bv, out = io["bq"], io["bk"], io["bv"], io["out"]

    with ExitStack() as ctx:
        const = ctx.enter_context(tc.tile_pool(name="const", bufs=1))
        big = ctx.enter_context(tc.tile_pool(name="big", bufs=1))
        epool = ctx.enter_context(tc.tile_pool(name="epool", bufs=2))
        opool = ctx.enter_context(tc.tile_pool(name="opool", bufs=2))
        small = ctx.enter_context(tc.tile_pool(name="small", bufs=4))
        # "ps" slots are (128,1024) = 2 banks each, bufs=2 -> 4 banks.
        pp = ctx.enter_context(tc.tile_pool(name="pp", bufs=2, space="PSUM"))
        # O_T accumulator: (128,2048) = 4 banks, single slot.
        po = ctx.enter_context(tc.tile_pool(name="po", bufs=1, space="PSUM"))

        # ---------- constants + inputs ----------
        xt_sb = big.tile([128, EC, T], fp32, tag="xt")
        for c in range(EC):
            nc.sync.dma_start(out=xt_sb[:, c, :], in_=x_t[c * 128:(c + 1) * 128, :])

        wq_sb = const.tile([128, EC, 256], fp32, tag="wq")
        wk_sb = const.tile([128, EC, 256], fp32, tag="wk")
        wv_sb = const.tile([128, EC, 256], fp32, tag="wv")
        for c in range(EC):
            nc.sync.dma_start(out=wq_sb[:, c, :], in_=wq[c * 128:(c + 1) * 128, :])
            nc.sync.dma_start(out=wk_sb[:, c, :], in_=wk[c * 128:(c + 1) * 128, :])
            nc.sync.dma_start(out=wv_sb[:, c, :], in_=wv[c * 128:(c + 1) * 128, :])

        bq_sb = const.tile([128, PAIRS], fp32, tag="bq")
        bk_sb = const.tile([128, PAIRS], fp32, tag="bk")
        for p in range(PAIRS):
            nc.sync.dma_start(out=bq_sb[:, p:p + 1], in_=bq[p, :, None])
            nc.sync.dma_start(out=bk_sb[:, p:p + 1], in_=bk[p, :, None])

        bv_sb = const.tile([128, 256], fp32, tag="bv")
        bv_bcast = bass.AP(tensor=bv.tensor, offset=bv.offset,
                           ap=[[0, 128]] + list(bv.ap))
        nc.gpsimd.dma_start(out=bv_sb, in_=bv_bcast)

        # zeros row used to open O_T accumulation groups bank-wide
        zrow_sb = const.tile([1, 512], fp32, tag="zrow")
        nc.vector.memset(zrow_sb, 0.0)

        # mask[s, t] = 0 if t >= s else NEG   (s = partition, t = free)
        mask_sb = const.tile([128, 128], fp32, tag="mask")
        nc.vector.memset(mask_sb, 0.0)
        nc.gpsimd.affine_select(
            out=mask_sb, in_=mask_sb,
            pattern=[[1, 128]], channel_multiplier=-1, base=0,
            compare_op=ALU.is_ge, fill=NEG,
        )

        # ---------- phase B: projections ----------
        qt_sb = [big.tile([128, T], fp32, tag=f"qt{p}", name=f"qt{p}") for p in range(PAIRS)]
        kt_sb = [big.tile([128, T], fp32, tag=f"kt{p}", name=f"kt{p}") for p in range(PAIRS)]
        v_sb = big.tile([128, TB, 256], fp32, tag="v")

        for p in range(PAIRS):
            for dst, w_sb, b_sb in ((qt_sb[p], wq_sb, bq_sb),
                                    (kt_sb[p], wk_sb, bk_sb)):
                for j in range(TC):
                    ps = pp.tile([128, 1024], fp32, tag="ps")
                    for c in range(EC):
                        nc.tensor.matmul(
                            ps[:, 0:512],
                            lhsT=w_sb[:, c, 128 * p:128 * (p + 1)],
                            rhs=xt_sb[:, c, 512 * j:512 * (j + 1)],
                            start=(c == 0), stop=(c == EC - 1),
                        )
                    # PSUM -> SBUF with per-partition bias add
                    nc.vector.tensor_scalar_add(
                        out=dst[:, 512 * j:512 * (j + 1)],
                        in0=ps[:, 0:512], scalar1=b_sb[:, p:p + 1],
                    )

        for tb in range(TB):
            ps = pp.tile([128, 1024], fp32, tag="ps")
            for c in range(EC):
                nc.tensor.matmul(
                    ps[:, 0:256],
                    lhsT=xt_sb[:, c, 128 * tb:128 * (tb + 1)],
                    rhs=wv_sb[:, c, :],
                    start=(c == 0), stop=(c == EC - 1),
                )
            nc.vector.tensor_tensor(out=v_sb[:, tb, :], in0=ps[:, 0:256],
                                    in1=bv_sb, op=ALU.add)

        # ---------- phase C: attention ----------
        for p in range(PAIRS):
            po_t = po.tile([128, T], fp32, tag="po")
            # Zero each O_T bank with one all-partition k=1 matmul so every
            # later AV matmul is a pure accumulation (first_mm=0) -- immune to
            # whole-bank has_written clears.
            for j in range(TC):
                nc.tensor.matmul(
                    po_t[:, 512 * j:512 * (j + 1)],
                    lhsT=zrow_sb[0:1, 0:128], rhs=zrow_sb[0:1, 0:512],
                    start=True, stop=False, skip_group_check=True,
                )
            for i in range(TB):
                s0 = 128 * i
                halves = [(h0, min(T, h0 + 1024))
                          for h0 in (s0, s0 + 1024) if h0 < T]
                e_t = [epool.tile([128, T], fp32, tag=f"e{h}", name=f"e{h}") for h in range(2)]
                den = small.tile([128, 2, 2], fp32, tag="den")
                rec = small.tile([128, 2], fp32, tag="rec")
                vp = small.tile([128, 128], fp32, tag="vp")

                for h in range(2):
                    lhsT_k = kt_sb[p][64 * h:64 * (h + 1), s0:s0 + 128]
                    rhs_q = qt_sb[p][64 * h:64 * (h + 1), :]
                    for hf, (h0, h1) in enumerate(halves):
                        wh = h1 - h0
                        ps = pp.tile([128, 1024], fp32, tag="ps")
                        for c0, cw in _st_chunks(h0, h1):
                            nc.tensor.matmul(
                                ps[:, c0 - h0:c0 - h0 + cw],
                                lhsT=lhsT_k, rhs=rhs_q[:, c0:c0 + cw],
                                start=True, stop=True,
                            )
                        if hf == 0:
                            nc.vector.tensor_tensor(
                                out=ps[:, 0:128], in0=ps[:, 0:128],
                                in1=mask_sb, op=ALU.add)
                        nc.scalar.activation(
                            out=e_t[h][:, h0 - s0:h0 - s0 + wh],
                            in_=ps[:, 0:wh], func=AF.Exp, scale=SCALE,
                            accum_out=den[:, h, hf:hf + 1],
                        )
                    if len(halves) == 2:
                        nc.vector.tensor_reduce(
                            out=rec[:, h:h + 1], in_=den[:, h, :],
                            axis=mybir.AxisListType.X, op=ALU.add)
                        nc.vector.reciprocal(rec[:, h:h + 1], rec[:, h:h + 1])
                    else:
                        nc.vector.reciprocal(rec[:, h:h + 1], den[:, h, 0:1])
                    nc.vector.tensor_scalar_mul(
                        out=vp[:, 64 * h:64 * (h + 1)],
                        in0=v_sb[:, i, 128 * p + 64 * h:128 * p + 64 * (h + 1)],
                        scalar1=rec[:, h:h + 1],
                    )
                for h in range(2):
                    for j, c0, cw in _av_chunks(s0):
                        nc.tensor.matmul(
                            po_t[64 * h:64 * (h + 1), c0:c0 + cw],
                            lhsT=vp[:, 64 * h:64 * (h + 1)],
                            rhs=e_t[h][:, c0 - s0:c0 - s0 + cw],
                            start=False, stop=(i == min(TB - 1, 4 * j + 3)),
                            tile_position=(0, 64 * h),
                            skip_group_check=True,
                        )

            o_sb = opool.tile([128, T], fp32, tag="o")
            for j in range(TC):
                src = po_t[:, 512 * j:512 * (j + 1)]
                dst = o_sb[:, 512 * j:512 * (j + 1)]
                if j % 2 == 0:
                    nc.scalar.copy(dst, src)
                else:
                    nc.vector.tensor_copy(dst, src)
            nc.sync.dma_start(out=out[p], in_=o_sb)


def _build():
    """Build + schedule + compile the (SPMD-identical) program once."""
    if "nc" in _CACHE:
        return _CACHE["nc"]
    import concourse.bacc as bacc
    import concourse.mybir as mybir
    import concourse.tile as tile

    fp32 = mybir.dt.float32
    nc = bacc.Bacc("TRN2", target_bir_lowering=False, debug=False)
    io = {
        "x_t": nc.dram_tensor("x_t", [E, T], fp32, kind="ExternalInput").ap(),
        "wq": nc.dram_tensor("wq", [E, 256], fp32, kind="ExternalInput").ap(),
        "wk": nc.dram_tensor("wk", [E, 256], fp32, kind="ExternalInput").ap(),
        "wv": nc.dram_tensor("wv", [E, 256], fp32, kind="ExternalInput").ap(),
        "bq": nc.dram_tensor("bq", [PAIRS, 128], fp32, kind="ExternalInput").ap(),
        "bk": nc.dram_tensor("bk", [PAIRS, 128], fp32, kind="ExternalInput").ap(),
        "bv": nc.dram_tensor("bv", [256], fp32, kind="ExternalInput").ap(),
        "out": nc.dram_tensor("out", [PAIRS, 128, T], fp32,
                              kind="ExternalOutput").ap(),
    }
    with tile.TileContext(nc) as tc:
        _emit(tc, io)
    nc.compile()
    _CACHE["nc"] = nc
    return nc


def make_in_maps(inputs_x, k_w, k_b, q_w, q_b, v_w, v_b):
    """Host-side sharding: per-core input dict."""
    xs = [np.ascontiguousarray(inputs_x[b].T.astype(np.float32, copy=False))
          for b in range(B)]
    in_maps = []
    for core in range(NCORES):
        b, g = divmod(core, 4)
        hs = range(4 * g, 4 * g + 4)
        pack_w = lambda w: np.ascontiguousarray(
            np.concatenate([w[h] for h in hs], axis=1).astype(np.float32, copy=False))
        pack_b2 = lambda bb: np.ascontiguousarray(
            bb[4 * g:4 * g + 4].reshape(PAIRS, 128).astype(np.float32, copy=False))
        in_maps.append({
            "x_t": xs[b],
            "wq": pack_w(q_w), "wk": pack_w(k_w), "wv": pack_w(v_w),
            "bq": pack_b2(q_b), "bk": pack_b2(k_b),
            "bv": np.ascontiguousarray(
                v_b[4 * g:4 * g + 4].reshape(256).astype(np.float32, copy=False)),
        })
    return in_maps


def assemble(core_outs):
    """Gather per-core (PAIRS, 128, T) outputs into the full (B, T, H*D)."""
    out_full = np.empty((B, T, H * D), np.float32)
    for core in range(NCORES):
        b, g = divmod(core, 4)
        o = core_outs[core]
        for p in range(PAIRS):
            out_full[b, :, g * 256 + 128 * p:g * 256 + 128 * (p + 1)] = o[p].T
    return out_full


def kernel(**inputs):
    x = np.asarray(inputs["inputs"], np.float32)
    args = [np.asarray(inputs[k], np.float32)
            for k in ("k_w", "k_b", "q_w", "q_b", "v_w", "v_b")]
    from concourse.bass_utils import run_bass_kernel_spmd

    nc = _build()
    in_maps = make_in_maps(x, *args)
    res = run_bass_kernel_spmd(nc, in_maps, core_ids=list(range(NCORES)))
    return assemble([r["out"] for r in res.results])


# revision 6
# speedup vs baseline: 2.0150x; 2.0150x over previous
"""Trainium2 Bass kernel: causal multi-head attention with softmax over the
QUERY axis (faithful to the reference's softmax(dim=-2) quirk).

Problem shapes: B=2, T=2048, E=1024, H=16, D=64.

Sharding: 8 cores = 2 batches x 4 head-groups (4 heads per core, zero
communication).  Host pre-transposes X to (E, T) per batch, packs per-head
weights into head-pair columns, and reassembles the output from per-core
(2, 128, 2048) tiles.

Per-core math (heads as two pairs (hA, hB)):
  Q_T, K_T: (d, t) layout via weight-stationary matmuls (2 heads packed into
            the 128-partition dim), bias added during PSUM->SBUF copy.
  V:        (t, d) layout via X_T-stationary matmuls (all 4 heads packed into
            the free dim), bias broadcast-added by DVE.
  S_T = K_T^T Q_T in (s, t) layout, computed only for t >= s0 (causal), so the
            softmax-over-queries is a free-axis reduction.  Triangular -1e30
            mask added on the 128-wide diagonal block only.
  exp on ACT with fused accum_out => denominators come free.
  Normalize V (2048x64 per head) by 1/denom instead of the O(T^2) weights.
  O_T accumulated in PSUM via V'-stationary matmuls (2 heads packed via PE
            column tiling), copied out and DMA'd as (d, t) rows.
"""

import numpy as np
from contextlib import ExitStack

B, T, E, H, D = 2, 2048, 1024, 16, 64
NCORES = 8
PAIRS = 2          # head pairs per core (4 heads)
EC = E // 128      # 8 contraction chunks
TB = T // 128      # 16 s-blocks
TC = T // 512      # 4 output column chunks
NEG = -1.0e30
SCALE = float(D) ** -0.5

_CACHE = {}


def _st_chunks(h0, h1):
    """Split [h0, h1) into <=512 pieces aligned to the psum tile's own 512
    grid (tile column 0 is at absolute t=h0)."""
    out = []
    c0 = h0
    while c0 < h1:
        w = min(512, h1 - c0)
        out.append((c0, w))
        c0 += w
    return out


def _av_chunks(s0):
    """Split [s0, 2048) on the absolute 512 grid (psum_o bank alignment)."""
    out = []
    for j in range(s0 // 512, TC):
        c0 = max(s0, 512 * j)
        out.append((j, c0, 512 * (j + 1) - c0))
    return out


def _emit(tc, io):
    """Emit the kernel program into TileContext tc.  io: dict name -> AP."""
    import concourse.bass as bass
    import concourse.mybir as mybir

    nc = tc.nc
    fp32 = mybir.dt.float32
    fp16 = mybir.dt.float16
    AF = mybir.ActivationFunctionType
    ALU = mybir.AluOpType

    x_t, wq, wk, wv = io["x_t"], io["wq"], io["wk"], io["wv"]
    bq, bk, bv, out = io["bq"], io["bk"], io["bv"], io["out"]

    with ExitStack() as ctx:
        const = ctx.enter_context(tc.tile_pool(name="const", bufs=1))
        big = ctx.enter_context(tc.tile_pool(name="big", bufs=1))
        epool = ctx.enter_context(tc.tile_pool(name="epool", bufs=2))
        opool = ctx.enter_context(tc.tile_pool(name="opool", bufs=2))
        small = ctx.enter_context(tc.tile_pool(name="small", bufs=4))
        # "ps" slots are (128,1024) = 2 banks each, bufs=2 -> 4 banks.
        pp = ctx.enter_context(tc.tile_pool(name="pp", bufs=2, space="PSUM"))
        # O_T accumulator: (128,2048) = 4 banks, single slot.
        po = ctx.enter_context(tc.tile_pool(name="po", bufs=1, space="PSUM"))

        # ---------- constants + inputs ----------
        xt_sb = big.tile([128, EC, T], fp16, tag="xt")
        for c in range(EC):
            nc.sync.dma_start(out=xt_sb[:, c, :], in_=x_t[c * 128:(c + 1) * 128, :])

        wq_sb = const.tile([128, EC, 256], fp16, tag="wq")
        wk_sb = const.tile([128, EC, 256], fp16, tag="wk")
        wv_sb = const.tile([128, EC, 256], fp16, tag="wv")
        for c in range(EC):
            nc.sync.dma_start(out=wq_sb[:, c, :], in_=wq[c * 128:(c + 1) * 128, :])
            nc.sync.dma_start(out=wk_sb[:, c, :], in_=wk[c * 128:(c + 1) * 128, :])
            nc.sync.dma_start(out=wv_sb[:, c, :], in_=wv[c * 128:(c + 1) * 128, :])

        bq_sb = const.tile([128, PAIRS], fp32, tag="bq")
        bk_sb = const.tile([128, PAIRS], fp32, tag="bk")
        for p in range(PAIRS):
            nc.sync.dma_start(out=bq_sb[:, p:p + 1], in_=bq[p, :, None])
            nc.sync.dma_start(out=bk_sb[:, p:p + 1], in_=bk[p, :, None])

        bv_sb = const.tile([128, 256], fp32, tag="bv")
        bv_bcast = bass.AP(tensor=bv.tensor, offset=bv.offset,
                           ap=[[0, 128]] + list(bv.ap))
        nc.gpsimd.dma_start(out=bv_sb, in_=bv_bcast)

        # zeros row used to open O_T accumulation groups bank-wide
        zrow_sb = const.tile([1, 512], fp16, tag="zrow")
        nc.vector.memset(zrow_sb, 0.0)

        # mask[s, t] = 0 if t >= s else NEG   (s = partition, t = free)
        mask_sb = const.tile([128, 128], fp32, tag="mask")
        nc.vector.memset(mask_sb, 0.0)
        nc.gpsimd.affine_select(
            out=mask_sb, in_=mask_sb,
            pattern=[[1, 128]], channel_multiplier=-1, base=0,
            compare_op=ALU.is_ge, fill=NEG,
        )

        # ---------- phase B: projections ----------
        qt_sb = [big.tile([128, T], fp16, tag=f"qt{p}", name=f"qt{p}") for p in range(PAIRS)]
        kt_sb = [big.tile([128, T], fp16, tag=f"kt{p}", name=f"kt{p}") for p in range(PAIRS)]
        v_sb = big.tile([128, TB, 256], fp16, tag="v")

        for p in range(PAIRS):
            for dst, w_sb, b_sb in ((qt_sb[p], wq_sb, bq_sb),
                                    (kt_sb[p], wk_sb, bk_sb)):
                for j in range(TC):
                    ps = pp.tile([128, 1024], fp32, tag="ps")
                    for c in range(EC):
                        nc.tensor.matmul(
                            ps[:, 0:512],
                            lhsT=w_sb[:, c, 128 * p:128 * (p + 1)],
                            rhs=xt_sb[:, c, 512 * j:512 * (j + 1)],
                            start=(c == 0), stop=(c == EC - 1),
                        )
                    # PSUM -> SBUF with per-partition bias add
                    nc.vector.tensor_scalar_add(
                        out=dst[:, 512 * j:512 * (j + 1)],
                        in0=ps[:, 0:512], scalar1=b_sb[:, p:p + 1],
                    )

        for tb in range(TB):
            ps = pp.tile([128, 1024], fp32, tag="ps")
            for c in range(EC):
                nc.tensor.matmul(
                    ps[:, 0:256],
                    lhsT=xt_sb[:, c, 128 * tb:128 * (tb + 1)],
                    rhs=wv_sb[:, c, :],
                    start=(c == 0), stop=(c == EC - 1),
                )
            nc.vector.tensor_tensor(out=v_sb[:, tb, :], in0=ps[:, 0:256],
                                    in1=bv_sb, op=ALU.add)

        # ---------- phase C: attention ----------
        for p in range(PAIRS):
            po_t = po.tile([128, T], fp32, tag="po")
            # Zero each O_T bank with one all-partition k=1 matmul so every
            # later AV matmul is a pure accumulation (first_mm=0) -- immune to
            # whole-bank has_written clears.
            for j in range(TC):
                nc.tensor.matmul(
                    po_t[:, 512 * j:512 * (j + 1)],
                    lhsT=zrow_sb[0:1, 0:128], rhs=zrow_sb[0:1, 0:512],
                    start=True, stop=False, skip_group_check=True,
                )
            for i in range(TB):
                s0 = 128 * i
                halves = [(h0, min(T, h0 + 1024))
                          for h0 in (s0, s0 + 1024) if h0 < T]
                e_t = [epool.tile([128, T], fp16, tag=f"e{h}", name=f"e{h}") for h in range(2)]
                den = small.tile([128, 2, 2], fp32, tag="den")
                rec = small.tile([128, 2], fp32, tag="rec")
                vp = small.tile([128, 128], fp16, tag="vp")

                for h in range(2):
                    lhsT_k = kt_sb[p][64 * h:64 * (h + 1), s0:s0 + 128]
                    rhs_q = qt_sb[p][64 * h:64 * (h + 1), :]
                    for hf, (h0, h1) in enumerate(halves):
                        wh = h1 - h0
                        ps = pp.tile([128, 1024], fp32, tag="ps")
                        for c0, cw in _st_chunks(h0, h1):
                            nc.tensor.matmul(
                                ps[:, c0 - h0:c0 - h0 + cw],
                                lhsT=lhsT_k, rhs=rhs_q[:, c0:c0 + cw],
                                start=True, stop=True,
                            )
                        if hf == 0:
                            nc.vector.tensor_tensor(
                                out=ps[:, 0:128], in0=ps[:, 0:128],
                                in1=mask_sb, op=ALU.add)
                        nc.scalar.activation(
                            out=e_t[h][:, h0 - s0:h0 - s0 + wh],
                            in_=ps[:, 0:wh], func=AF.Exp, scale=SCALE,
                            accum_out=den[:, h, hf:hf + 1],
                        )
                    if len(halves) == 2:
                        nc.vector.tensor_reduce(
                            out=rec[:, h:h + 1], in_=den[:, h, :],
                            axis=mybir.AxisListType.X, op=ALU.add)
                        nc.vector.reciprocal(rec[:, h:h + 1], rec[:, h:h + 1])
                    else:
                        nc.vector.reciprocal(rec[:, h:h + 1], den[:, h, 0:1])
                    nc.vector.tensor_scalar_mul(
                        out=vp[:, 64 * h:64 * (h + 1)],
                        in0=v_sb[:, i, 128 * p + 64 * h:128 * p + 64 * (h + 1)],
                        scalar1=rec[:, h:h + 1],
                    )
                for h in range(2):
                    for j, c0, cw in _av_chunks(s0):
                        nc.tensor.matmul(
                            po_t[64 * h:64 * (h + 1), c0:c0 + cw],
                            lhsT=vp[:, 64 * h:64 * (h + 1)],
                            rhs=e_t[h][:, c0 - s0:c0 - s0 + cw],
                            start=False, stop=(i == min(TB - 1, 4 * j + 3)),
                            tile_position=(0, 64 * h),
                            skip_group_check=True,
                        )

            o_sb = opool.tile([128, T], fp32, tag="o")
            for j in range(TC):
                src = po_t[:, 512 * j:512 * (j + 1)]
                dst = o_sb[:, 512 * j:512 * (j + 1)]
                if j % 2 == 0:
                    nc.scalar.copy(dst, src)
                else:
                    nc.vector.tensor_copy(dst, src)
            nc.sync.dma_start(out=out[p], in_=o_sb)


def _build():
    """Build + schedule + compile the (SPMD-identical) program once."""
    if "nc" in _CACHE:
        return _CACHE["nc"]
    import concourse.bacc as bacc
    import concourse.mybir as mybir
    import concourse.tile as tile

    fp32 = mybir.dt.float32
    fp16 = mybir.dt.float16
    nc = bacc.Bacc("TRN2", target_bir_lowering=False, debug=False)
    io = {
        "x_t": nc.dram_tensor("x_t", [E, T], fp16, kind="ExternalInput").ap(),
        "wq": nc.dram_tensor("wq", [E, 256], fp16, kind="ExternalInput").ap(),
        "wk": nc.dram_tensor("wk", [E, 256], fp16, kind="ExternalInput").ap(),
        "wv": nc.dram_tensor("wv", [E, 256], fp16, kind="ExternalInput").ap(),
        "bq": nc.dram_tensor("bq", [PAIRS, 128], fp32, kind="ExternalInput").ap(),
        "bk": nc.dram_tensor("bk", [PAIRS, 128], fp32, kind="ExternalInput").ap(),
        "bv": nc.dram_tensor("bv", [256], fp32, kind="ExternalInput").ap(),
        "out": nc.dram_tensor("out", [PAIRS, 128, T], fp32,
                              kind="ExternalOutput").ap(),
    }
    with tile.TileContext(nc) as tc:
        _emit(tc, io)
    nc.compile()
    _CACHE["nc"] = nc
    return nc


def make_in_maps(inputs_x, k_w, k_b, q_w, q_b, v_w, v_b):
    """Host-side sharding: per-core input dict."""
    xs = [np.ascontiguousarray(inputs_x[b].T.astype(np.float16))
          for b in range(B)]
    in_maps = []
    for core in range(NCORES):
        b, g = divmod(core, 4)
        hs = range(4 * g, 4 * g + 4)
        pack_w = lambda w: np.ascontiguousarray(
            np.concatenate([w[h] for h in hs], axis=1).astype(np.float16))
        pack_b2 = lambda bb: np.ascontiguousarray(
            bb[4 * g:4 * g + 4].reshape(PAIRS, 128).astype(np.float32, copy=False))
        in_maps.append({
            "x_t": xs[b],
            "wq": pack_w(q_w), "wk": pack_w(k_w), "wv": pack_w(v_w),
            "bq": pack_b2(q_b), "bk": pack_b2(k_b),
            "bv": np.ascontiguousarray(
                v_b[4 * g:4 * g + 4].reshape(256).astype(np.float32, copy=False)),
        })
    return in_maps


def assemble(core_outs):
    """Gather per-core (PAIRS, 128, T) outputs into the full (B, T, H*D)."""
    out_full = np.empty((B, T, H * D), np.float32)
    for core in range(NCORES):
        b, g = divmod(core, 4)
        o = core_outs[core]
        for p in range(PAIRS):
            out_full[b, :, g * 256 + 128 * p:g * 256 + 128 * (p + 1)] = o[p].T
    return out_full


def kernel(**inputs):
    x = np.asarray(inputs["inputs"], np.float32)
    args = [np.asarray(inputs[k], np.float32)
            for k in ("k_w", "k_b", "q_w", "q_b", "v_w", "v_b")]
    from concourse.bass_utils import run_bass_kernel_spmd

    nc = _build()
    in_maps = make_in_maps(x, *args)
    res = run_bass_kernel_spmd(nc, in_maps, core_ids=list(range(NCORES)))
    return assemble([r["out"] for r in res.results])


# revision 11
# speedup vs baseline: 2.0436x; 1.0142x over previous
"""Trainium2 Bass kernel: causal multi-head attention with softmax over the
QUERY axis (faithful to the reference's softmax(dim=-2) quirk).

Problem shapes: B=2, T=2048, E=1024, H=16, D=64.

Sharding: 8 cores = 2 batches x 4 head-groups (4 heads per core, zero
communication).  Host pre-transposes X to (E, T) per batch, packs per-head
weights into head-pair columns, and reassembles the output from per-core
(2, 128, 2048) tiles.

Per-core math (heads as two pairs (hA, hB)):
  Q_T, K_T: (d, t) layout via weight-stationary matmuls (2 heads packed into
            the 128-partition dim), bias added during PSUM->SBUF copy.
  V:        (t, d) layout via X_T-stationary matmuls (all 4 heads packed into
            the free dim), bias broadcast-added by DVE.
  S_T = K_T^T Q_T in (s, t) layout, computed only for t >= s0 (causal), so the
            softmax-over-queries is a free-axis reduction.  Triangular -1e30
            mask added on the 128-wide diagonal block only.
  exp on ACT with fused accum_out => denominators come free.
  Normalize V (2048x64 per head) by 1/denom instead of the O(T^2) weights.
  O_T accumulated in PSUM via V'-stationary matmuls (2 heads packed via PE
            column tiling), copied out and DMA'd as (d, t) rows.
"""

import numpy as np
from contextlib import ExitStack

B, T, E, H, D = 2, 2048, 1024, 16, 64
NCORES = 8
PAIRS = 2          # head pairs per core (4 heads)
EC = E // 128      # 8 contraction chunks
TB = T // 128      # 16 s-blocks
TC = T // 512      # 4 output column chunks
NEG = -1.0e30
SCALE = float(D) ** -0.5

_CACHE = {}


def _st_chunks(h0, h1):
    """Split [h0, h1) into <=512 pieces aligned to the psum tile's own 512
    grid (tile column 0 is at absolute t=h0)."""
    out = []
    c0 = h0
    while c0 < h1:
        w = min(512, h1 - c0)
        out.append((c0, w))
        c0 += w
    return out


def _av_chunks(s0):
    """Split [s0, 2048) on the absolute 512 grid (psum_o bank alignment)."""
    out = []
    for j in range(s0 // 512, TC):
        c0 = max(s0, 512 * j)
        out.append((j, c0, 512 * (j + 1) - c0))
    return out


def _emit(tc, io):
    """Emit the kernel program into TileContext tc.  io: dict name -> AP."""
    import concourse.bass as bass
    import concourse.mybir as mybir

    nc = tc.nc
    fp32 = mybir.dt.float32
    fp16 = mybir.dt.float16
    AF = mybir.ActivationFunctionType
    ALU = mybir.AluOpType

    x_t, wq, wk, wv = io["x_t"], io["wq"], io["wk"], io["wv"]
    bq, bk, bv, out = io["bq"], io["bk"], io["bv"], io["out"]

    with ExitStack() as ctx:
        const = ctx.enter_context(tc.tile_pool(name="const", bufs=1))
        big = ctx.enter_context(tc.tile_pool(name="big", bufs=1))
        epool = ctx.enter_context(tc.tile_pool(name="epool", bufs=2))
        opool = ctx.enter_context(tc.tile_pool(name="opool", bufs=2))
        small = ctx.enter_context(tc.tile_pool(name="small", bufs=4))
        # "ps" slots are (128,1024) = 2 banks each, bufs=2 -> 4 banks.
        pp = ctx.enter_context(tc.tile_pool(name="pp", bufs=2, space="PSUM"))
        # O_T accumulator: (128,2048) = 4 banks, single slot.
        po = ctx.enter_context(tc.tile_pool(name="po", bufs=1, space="PSUM"))

        # ---------- constants + inputs (weights first: compute starts early) ----------
        wq_sb = const.tile([128, EC, 256], fp16, tag="wq")
        wk_sb = const.tile([128, EC, 256], fp16, tag="wk")
        wv_sb = const.tile([128, EC, 256], fp16, tag="wv")
        for c in range(EC):
            nc.sync.dma_start(out=wq_sb[:, c, :], in_=wq[c * 128:(c + 1) * 128, :])
            nc.sync.dma_start(out=wk_sb[:, c, :], in_=wk[c * 128:(c + 1) * 128, :])
            nc.sync.dma_start(out=wv_sb[:, c, :], in_=wv[c * 128:(c + 1) * 128, :])

        bq_sb = const.tile([128, PAIRS], fp32, tag="bq")
        bk_sb = const.tile([128, PAIRS], fp32, tag="bk")
        for p in range(PAIRS):
            nc.sync.dma_start(out=bq_sb[:, p:p + 1], in_=bq[p, :, None])
            nc.sync.dma_start(out=bk_sb[:, p:p + 1], in_=bk[p, :, None])

        bv_sb = const.tile([128, 256], fp32, tag="bv")
        bv_bcast = bass.AP(tensor=bv.tensor, offset=bv.offset,
                           ap=[[0, 128]] + list(bv.ap))
        nc.gpsimd.dma_start(out=bv_sb, in_=bv_bcast)

        xt_sb = big.tile([128, EC, T], fp16, tag="xt")
        for c in range(EC):
            nc.sync.dma_start(out=xt_sb[:, c, :], in_=x_t[c * 128:(c + 1) * 128, :])

        # zeros row used to open O_T accumulation groups bank-wide
        zrow_sb = const.tile([1, 512], fp16, tag="zrow")
        nc.vector.memset(zrow_sb, 0.0)

        # mask[s, t] = 0 if t >= s else MNEG, applied on PE as identity @ mask
        MNEG = -60000.0
        mask_sb = const.tile([128, 128], fp16, tag="mask")
        nc.vector.memset(mask_sb, 0.0)
        nc.gpsimd.affine_select(
            out=mask_sb, in_=mask_sb,
            pattern=[[1, 128]], channel_multiplier=-1, base=0,
            compare_op=ALU.is_ge, fill=MNEG,
        )
        ident_sb = const.tile([128, 128], fp16, tag="ident")
        nc.vector.memset(ident_sb, 0.0)
        nc.gpsimd.affine_select(
            out=ident_sb, in_=ident_sb,
            pattern=[[1, 128]], channel_multiplier=-1, base=0,
            compare_op=ALU.not_equal, fill=1.0,
        )

        # ---------- phase B: projections ----------
        qt_sb = [big.tile([128, T], fp16, tag=f"qt{p}", name=f"qt{p}") for p in range(PAIRS)]
        kt_sb = [big.tile([128, T], fp16, tag=f"kt{p}", name=f"kt{p}") for p in range(PAIRS)]
        v_sb = big.tile([128, TB, 256], fp16, tag="v")

        def emit_v_blocks(tbs):
            for tb in tbs:
                ps = pp.tile([128, 1024], fp32, tag="ps", name="ps_v")
                for c in range(EC):
                    nc.tensor.matmul(
                        ps[:, 0:256],
                        lhsT=xt_sb[:, c, 128 * tb:128 * (tb + 1)],
                        rhs=wv_sb[:, c, :],
                        start=(c == 0), stop=(c == EC - 1),
                    )
                nc.vector.tensor_tensor(out=v_sb[:, tb, :], in0=ps[:, 0:256],
                                        in1=bv_sb, op=ALU.add)

        # Q/K: c-outer over a 4-bank scratch (the po pool, idle in phase B):
        # one weight load per (proj-pair, e-chunk), X_T consumed as it streams.
        vi = 0
        for p in range(PAIRS):
            for dst, w_sb, b_sb in ((qt_sb[p], wq_sb, bq_sb),
                                    (kt_sb[p], wk_sb, bk_sb)):
                sc = po.tile([128, T], fp32, tag="po", name="sc_qk")
                for c in range(EC):
                    for j in range(TC):
                        nc.tensor.matmul(
                            sc[:, 512 * j:512 * (j + 1)],
                            lhsT=w_sb[:, c, 128 * p:128 * (p + 1)],
                            rhs=xt_sb[:, c, 512 * j:512 * (j + 1)],
                            start=(c == 0), stop=(c == EC - 1),
                        )
                for j in range(TC):
                    # PSUM -> SBUF (fp16) with per-partition bias add on ACT
                    nc.scalar.activation(
                        out=dst[:, 512 * j:512 * (j + 1)],
                        in_=sc[:, 512 * j:512 * (j + 1)],
                        func=AF.Identity, bias=b_sb[:, p:p + 1], scale=1.0,
                    )
                emit_v_blocks(range(vi, vi + 4))
                vi += 4

        # ---------- phase C: attention ----------
        for p in range(PAIRS):
            po_t = po.tile([128, T], fp32, tag="po")
            # Zero each O_T bank with one all-partition k=1 matmul so every
            # later AV matmul is a pure accumulation (first_mm=0) -- immune to
            # whole-bank has_written clears.
            for j in range(TC):
                nc.tensor.matmul(
                    po_t[:, 512 * j:512 * (j + 1)],
                    lhsT=zrow_sb[0:1, 0:128], rhs=zrow_sb[0:1, 0:512],
                    start=True, stop=False, skip_group_check=True,
                )
            for i in range(TB):
                s0 = 128 * i
                halves = [(h0, min(T, h0 + 1024))
                          for h0 in (s0, s0 + 1024) if h0 < T]
                e_t = [epool.tile([128, T], fp16, tag=f"e{h}", name=f"e{h}") for h in range(2)]
                den = small.tile([128, 2, 2], fp32, tag="den")
                rec = small.tile([128, 2], fp32, tag="rec")
                vp = small.tile([128, 128], fp16, tag="vp")

                for h in range(2):
                    lhsT_k = kt_sb[p][64 * h:64 * (h + 1), s0:s0 + 128]
                    rhs_q = qt_sb[p][64 * h:64 * (h + 1), :]
                    for hf, (h0, h1) in enumerate(halves):
                        wh = h1 - h0
                        ps = pp.tile([128, 1024], fp32, tag="ps")
                        for c0, cw in _st_chunks(h0, h1):
                            diag = hf == 0 and c0 == h0
                            nc.tensor.matmul(
                                ps[:, c0 - h0:c0 - h0 + cw],
                                lhsT=lhsT_k, rhs=rhs_q[:, c0:c0 + cw],
                                start=True, stop=not diag,
                            )
                            if diag:
                                # causal mask add on PE: ps[:, :128] += I.T@mask
                                nc.tensor.matmul(
                                    ps[:, 0:128], lhsT=ident_sb, rhs=mask_sb,
                                    start=False, stop=True,
                                )
                        nc.scalar.activation(
                            out=e_t[h][:, h0 - s0:h0 - s0 + wh],
                            in_=ps[:, 0:wh], func=AF.Exp, scale=SCALE,
                            accum_out=den[:, h, hf:hf + 1],
                        )
                    if len(halves) == 2:
                        nc.vector.tensor_reduce(
                            out=rec[:, h:h + 1], in_=den[:, h, :],
                            axis=mybir.AxisListType.X, op=ALU.add)
                        nc.vector.reciprocal(rec[:, h:h + 1], rec[:, h:h + 1])
                    else:
                        nc.vector.reciprocal(rec[:, h:h + 1], den[:, h, 0:1])
                    nc.vector.tensor_scalar_mul(
                        out=vp[:, 64 * h:64 * (h + 1)],
                        in0=v_sb[:, i, 128 * p + 64 * h:128 * p + 64 * (h + 1)],
                        scalar1=rec[:, h:h + 1],
                    )
                for h in range(2):
                    for j, c0, cw in _av_chunks(s0):
                        nc.tensor.matmul(
                            po_t[64 * h:64 * (h + 1), c0:c0 + cw],
                            lhsT=vp[:, 64 * h:64 * (h + 1)],
                            rhs=e_t[h][:, c0 - s0:c0 - s0 + cw],
                            start=False, stop=(i == min(TB - 1, 4 * j + 3)),
                            tile_position=(0, 64 * h),
                            skip_group_check=True,
                        )

            o_sb = opool.tile([128, T], fp32, tag="o")
            for j in range(TC):
                nc.vector.tensor_copy(o_sb[:, 512 * j:512 * (j + 1)],
                                      po_t[:, 512 * j:512 * (j + 1)])
            nc.sync.dma_start(out=out[p], in_=o_sb)


def _build():
    """Build + schedule + compile the (SPMD-identical) program once."""
    if "nc" in _CACHE:
        return _CACHE["nc"]
    import concourse.bacc as bacc
    import concourse.mybir as mybir
    import concourse.tile as tile

    fp32 = mybir.dt.float32
    fp16 = mybir.dt.float16
    nc = bacc.Bacc("TRN2", target_bir_lowering=False, debug=False)
    io = {
        "x_t": nc.dram_tensor("x_t", [E, T], fp16, kind="ExternalInput").ap(),
        "wq": nc.dram_tensor("wq", [E, 256], fp16, kind="ExternalInput").ap(),
        "wk": nc.dram_tensor("wk", [E, 256], fp16, kind="ExternalInput").ap(),
        "wv": nc.dram_tensor("wv", [E, 256], fp16, kind="ExternalInput").ap(),
        "bq": nc.dram_tensor("bq", [PAIRS, 128], fp32, kind="ExternalInput").ap(),
        "bk": nc.dram_tensor("bk", [PAIRS, 128], fp32, kind="ExternalInput").ap(),
        "bv": nc.dram_tensor("bv", [256], fp32, kind="ExternalInput").ap(),
        "out": nc.dram_tensor("out", [PAIRS, 128, T], fp32,
                              kind="ExternalOutput").ap(),
    }
    with tile.TileContext(nc) as tc:
        _emit(tc, io)
    nc.compile()
    _CACHE["nc"] = nc
    return nc


def make_in_maps(inputs_x, k_w, k_b, q_w, q_b, v_w, v_b):
    """Host-side sharding: per-core input dict."""
    xs = [np.ascontiguousarray(inputs_x[b].T.astype(np.float16))
          for b in range(B)]
    in_maps = []
    for core in range(NCORES):
        b, g = divmod(core, 4)
        hs = range(4 * g, 4 * g + 4)
        pack_w = lambda w: np.ascontiguousarray(
            np.concatenate([w[h] for h in hs], axis=1).astype(np.float16))
        pack_b2 = lambda bb: np.ascontiguousarray(
            bb[4 * g:4 * g + 4].reshape(PAIRS, 128).astype(np.float32, copy=False))
        in_maps.append({
            "x_t": xs[b],
            "wq": pack_w(q_w), "wk": pack_w(k_w), "wv": pack_w(v_w),
            "bq": pack_b2(q_b), "bk": pack_b2(k_b),
            "bv": np.ascontiguousarray(
                v_b[4 * g:4 * g + 4].reshape(256).astype(np.float32, copy=False)),
        })
    return in_maps


def assemble(core_outs):
    """Gather per-core (PAIRS, 128, T) outputs into the full (B, T, H*D)."""
    out_full = np.empty((B, T, H * D), np.float32)
    for core in range(NCORES):
        b, g = divmod(core, 4)
        o = core_outs[core]
        for p in range(PAIRS):
            out_full[b, :, g * 256 + 128 * p:g * 256 + 128 * (p + 1)] = o[p].T
    return out_full


def kernel(**inputs):
    x = np.asarray(inputs["inputs"], np.float32)
    args = [np.asarray(inputs[k], np.float32)
            for k in ("k_w", "k_b", "q_w", "q_b", "v_w", "v_b")]
    from concourse.bass_utils import run_bass_kernel_spmd

    nc = _build()
    in_maps = make_in_maps(x, *args)
    res = run_bass_kernel_spmd(nc, in_maps, core_ids=list(range(NCORES)))
    return assemble([r["out"] for r in res.results])


# revision 13
# speedup vs baseline: 2.1564x; 1.0552x over previous
"""Trainium2 Bass kernel: causal multi-head attention with softmax over the
QUERY axis (faithful to the reference's softmax(dim=-2) quirk).

Problem shapes: B=2, T=2048, E=1024, H=16, D=64.

Sharding: 8 cores = 2 batches x 4 head-groups (4 heads per core, zero
communication).  Host pre-transposes X to (E, T) per batch, packs per-head
weights into head-pair columns, and reassembles the output from per-core
(2, 128, 2048) tiles.

Per-core math (heads as two pairs (hA, hB)):
  Q_T, K_T: (d, t) layout via weight-stationary matmuls (2 heads packed into
            the 128-partition dim), bias added during PSUM->SBUF copy.
  V:        (t, d) layout via X_T-stationary matmuls (all 4 heads packed into
            the free dim), bias broadcast-added by DVE.
  S_T = K_T^T Q_T in (s, t) layout, computed only for t >= s0 (causal), so the
            softmax-over-queries is a free-axis reduction.  Triangular -1e30
            mask added on the 128-wide diagonal block only.
  exp on ACT with fused accum_out => denominators come free.
  Normalize V (2048x64 per head) by 1/denom instead of the O(T^2) weights.
  O_T accumulated in PSUM via V'-stationary matmuls (2 heads packed via PE
            column tiling), copied out and DMA'd as (d, t) rows.
"""

import numpy as np
from contextlib import ExitStack

B, T, E, H, D = 2, 2048, 1024, 16, 64
NCORES = 8
PAIRS = 2          # head pairs per core (4 heads)
EC = E // 128      # 8 contraction chunks
TB = T // 128      # 16 s-blocks
TC = T // 512      # 4 output column chunks
NEG = -1.0e30
SCALE = float(D) ** -0.5

_CACHE = {}


def _st_chunks(h0, h1):
    """Split [h0, h1) into <=512 pieces aligned to the psum tile's own 512
    grid (tile column 0 is at absolute t=h0)."""
    out = []
    c0 = h0
    while c0 < h1:
        w = min(512, h1 - c0)
        out.append((c0, w))
        c0 += w
    return out


def _av_chunks(s0):
    """Split [s0, 2048) on the absolute 512 grid (psum_o bank alignment)."""
    out = []
    for j in range(s0 // 512, TC):
        c0 = max(s0, 512 * j)
        out.append((j, c0, 512 * (j + 1) - c0))
    return out


def _emit(tc, io):
    """Emit the kernel program into TileContext tc.  io: dict name -> AP."""
    import concourse.bass as bass
    import concourse.mybir as mybir

    nc = tc.nc
    fp32 = mybir.dt.float32
    fp16 = mybir.dt.float16
    AF = mybir.ActivationFunctionType
    ALU = mybir.AluOpType

    x_t, wq, wk, wv = io["x_t"], io["wq"], io["wk"], io["wv"]
    bq, bk, bv, out = io["bq"], io["bk"], io["bv"], io["out"]

    with ExitStack() as ctx:
        const = ctx.enter_context(tc.tile_pool(name="const", bufs=1))
        big = ctx.enter_context(tc.tile_pool(name="big", bufs=1))
        epool = ctx.enter_context(tc.tile_pool(name="epool", bufs=3))
        opool = ctx.enter_context(tc.tile_pool(name="opool", bufs=2))
        small = ctx.enter_context(tc.tile_pool(name="small", bufs=6))
        # "ps" slots are (128,1024) = 2 banks each, bufs=2 -> 4 banks.
        pp = ctx.enter_context(tc.tile_pool(name="pp", bufs=2, space="PSUM"))
        # O_T accumulator: (128,2048) = 4 banks, single slot.
        po = ctx.enter_context(tc.tile_pool(name="po", bufs=1, space="PSUM"))

        # ---------- constants + inputs (weights first: compute starts early) ----------
        wq_sb = const.tile([128, EC, 256], fp16, tag="wq")
        wk_sb = const.tile([128, EC, 256], fp16, tag="wk")
        wv_sb = const.tile([128, EC, 256], fp16, tag="wv")
        for c in range(EC):
            nc.sync.dma_start(out=wq_sb[:, c, :], in_=wq[c * 128:(c + 1) * 128, :])
            nc.sync.dma_start(out=wk_sb[:, c, :], in_=wk[c * 128:(c + 1) * 128, :])
            nc.sync.dma_start(out=wv_sb[:, c, :], in_=wv[c * 128:(c + 1) * 128, :])

        bq_sb = const.tile([128, PAIRS], fp32, tag="bq")
        bk_sb = const.tile([128, PAIRS], fp32, tag="bk")
        for p in range(PAIRS):
            nc.sync.dma_start(out=bq_sb[:, p:p + 1], in_=bq[p, :, None])
            nc.sync.dma_start(out=bk_sb[:, p:p + 1], in_=bk[p, :, None])

        bv_sb = const.tile([128, 256], fp32, tag="bv")
        bv_bcast = bass.AP(tensor=bv.tensor, offset=bv.offset,
                           ap=[[0, 128]] + list(bv.ap))
        nc.gpsimd.dma_start(out=bv_sb, in_=bv_bcast)

        xt_sb = big.tile([128, EC, T], fp16, tag="xt")
        for c in range(EC):
            nc.sync.dma_start(out=xt_sb[:, c, :], in_=x_t[c * 128:(c + 1) * 128, :])

        # zeros row used to open O_T accumulation groups bank-wide
        zrow_sb = const.tile([1, 512], fp16, tag="zrow")
        nc.vector.memset(zrow_sb, 0.0)

        # mask[s, t] = 0 if t >= s else MNEG, applied on PE as identity @ mask
        MNEG = -60000.0
        mask_sb = const.tile([128, 128], fp16, tag="mask")
        nc.vector.memset(mask_sb, 0.0)
        nc.gpsimd.affine_select(
            out=mask_sb, in_=mask_sb,
            pattern=[[1, 128]], channel_multiplier=-1, base=0,
            compare_op=ALU.is_ge, fill=MNEG,
        )
        ident_sb = const.tile([128, 128], fp16, tag="ident")
        nc.vector.memset(ident_sb, 0.0)
        nc.gpsimd.affine_select(
            out=ident_sb, in_=ident_sb,
            pattern=[[1, 128]], channel_multiplier=-1, base=0,
            compare_op=ALU.not_equal, fill=1.0,
        )

        # ---------- phase B: projections ----------
        qt_sb = [big.tile([128, T], fp16, tag=f"qt{p}", name=f"qt{p}") for p in range(PAIRS)]
        kt_sb = [big.tile([128, T], fp16, tag=f"kt{p}", name=f"kt{p}") for p in range(PAIRS)]
        v_sb = big.tile([128, TB, 256], fp16, tag="v")

        def emit_v_blocks(tbs):
            for tb in tbs:
                ps = pp.tile([128, 1024], fp32, tag="ps", name="ps_v")
                for c in range(EC):
                    nc.tensor.matmul(
                        ps[:, 0:256],
                        lhsT=xt_sb[:, c, 128 * tb:128 * (tb + 1)],
                        rhs=wv_sb[:, c, :],
                        start=(c == 0), stop=(c == EC - 1),
                    )
                nc.vector.tensor_tensor(out=v_sb[:, tb, :], in0=ps[:, 0:256],
                                        in1=bv_sb, op=ALU.add)

        # Q/K: c-outer over a 4-bank scratch (the po pool, idle in phase B):
        # one weight load per (proj-pair, e-chunk), X_T consumed as it streams.
        vi = 0
        for p in range(PAIRS):
            for dst, w_sb, b_sb in ((qt_sb[p], wq_sb, bq_sb),
                                    (kt_sb[p], wk_sb, bk_sb)):
                sc = po.tile([128, T], fp32, tag="po", name="sc_qk")
                for c in range(EC):
                    for j in range(TC):
                        nc.tensor.matmul(
                            sc[:, 512 * j:512 * (j + 1)],
                            lhsT=w_sb[:, c, 128 * p:128 * (p + 1)],
                            rhs=xt_sb[:, c, 512 * j:512 * (j + 1)],
                            start=(c == 0), stop=(c == EC - 1),
                        )
                for j in range(TC):
                    # PSUM -> SBUF (fp16) with per-partition bias add on ACT
                    nc.scalar.activation(
                        out=dst[:, 512 * j:512 * (j + 1)],
                        in_=sc[:, 512 * j:512 * (j + 1)],
                        func=AF.Identity, bias=b_sb[:, p:p + 1], scale=1.0,
                    )
                emit_v_blocks(range(vi, vi + 4))
                vi += 4

        # ---------- phase C: attention ----------
        for p in range(PAIRS):
            po_t = po.tile([128, T], fp32, tag="po")
            # Zero each O_T bank with one all-partition k=1 matmul so every
            # later AV matmul is a pure accumulation (first_mm=0) -- immune to
            # whole-bank has_written clears.
            for j in range(TC):
                nc.tensor.matmul(
                    po_t[:, 512 * j:512 * (j + 1)],
                    lhsT=zrow_sb[0:1, 0:128], rhs=zrow_sb[0:1, 0:512],
                    start=True, stop=False, skip_group_check=True,
                )
            def emit_st_exp(i):
                s0 = 128 * i
                halves = [(h0, min(T, h0 + 1024))
                          for h0 in (s0, s0 + 1024) if h0 < T]
                e_t = [epool.tile([128, T], fp16, tag=f"e{h}", name=f"e{h}")
                       for h in range(2)]
                den = small.tile([128, 2, 2], fp32, tag="den", name="den")
                for h in range(2):
                    lhsT_k = kt_sb[p][64 * h:64 * (h + 1), s0:s0 + 128]
                    rhs_q = qt_sb[p][64 * h:64 * (h + 1), :]
                    for hf, (h0, h1) in enumerate(halves):
                        wh = h1 - h0
                        ps = pp.tile([128, 1024], fp32, tag="ps", name="ps_st")
                        for c0, cw in _st_chunks(h0, h1):
                            diag = hf == 0 and c0 == h0
                            nc.tensor.matmul(
                                ps[:, c0 - h0:c0 - h0 + cw],
                                lhsT=lhsT_k, rhs=rhs_q[:, c0:c0 + cw],
                                start=True, stop=not diag,
                            )
                            if diag:
                                # causal mask add on PE: ps[:, :128] += I.T@mask
                                nc.tensor.matmul(
                                    ps[:, 0:128], lhsT=ident_sb, rhs=mask_sb,
                                    start=False, stop=True,
                                )
                        nc.scalar.activation(
                            out=e_t[h][:, h0 - s0:h0 - s0 + wh],
                            in_=ps[:, 0:wh], func=AF.Exp, scale=SCALE,
                            accum_out=den[:, h, hf:hf + 1],
                        )
                return i, len(halves), e_t, den

            def emit_norm_av(st):
                i, nhalves, e_t, den = st
                s0 = 128 * i
                rec = small.tile([128, 2], fp32, tag="rec", name="rec")
                vp = small.tile([128, 128], fp16, tag="vp", name="vp")
                for h in range(2):
                    if nhalves == 2:
                        nc.vector.tensor_reduce(
                            out=rec[:, h:h + 1], in_=den[:, h, :],
                            axis=mybir.AxisListType.X, op=ALU.add)
                        nc.vector.reciprocal(rec[:, h:h + 1], rec[:, h:h + 1])
                    else:
                        nc.vector.reciprocal(rec[:, h:h + 1], den[:, h, 0:1])
                    nc.vector.tensor_scalar_mul(
                        out=vp[:, 64 * h:64 * (h + 1)],
                        in0=v_sb[:, i, 128 * p + 64 * h:128 * p + 64 * (h + 1)],
                        scalar1=rec[:, h:h + 1],
                    )
                for h in range(2):
                    for j, c0, cw in _av_chunks(s0):
                        nc.tensor.matmul(
                            po_t[64 * h:64 * (h + 1), c0:c0 + cw],
                            lhsT=vp[:, 64 * h:64 * (h + 1)],
                            rhs=e_t[h][:, c0 - s0:c0 - s0 + cw],
                            start=False, stop=(i == min(TB - 1, 4 * j + 3)),
                            tile_position=(0, 64 * h),
                            skip_group_check=True,
                        )

            # Software pipeline: keep S_T/exp one block ahead of the
            # normalize+AV stage so the PE FIFO never blocks ACT behind the
            # DVE recip chain (AV accumulations are order-independent).
            pend = None
            for i in range(TB):
                st = emit_st_exp(i)
                if pend is not None:
                    emit_norm_av(pend)
                pend = st
            emit_norm_av(pend)

            o_sb = opool.tile([128, T], fp32, tag="o")
            for j in range(TC):
                nc.vector.tensor_copy(o_sb[:, 512 * j:512 * (j + 1)],
                                      po_t[:, 512 * j:512 * (j + 1)])
            nc.sync.dma_start(out=out[p], in_=o_sb)


def _build():
    """Build + schedule + compile the (SPMD-identical) program once."""
    if "nc" in _CACHE:
        return _CACHE["nc"]
    import concourse.bacc as bacc
    import concourse.mybir as mybir
    import concourse.tile as tile

    fp32 = mybir.dt.float32
    fp16 = mybir.dt.float16
    nc = bacc.Bacc("TRN2", target_bir_lowering=False, debug=False)
    io = {
        "x_t": nc.dram_tensor("x_t", [E, T], fp16, kind="ExternalInput").ap(),
        "wq": nc.dram_tensor("wq", [E, 256], fp16, kind="ExternalInput").ap(),
        "wk": nc.dram_tensor("wk", [E, 256], fp16, kind="ExternalInput").ap(),
        "wv": nc.dram_tensor("wv", [E, 256], fp16, kind="ExternalInput").ap(),
        "bq": nc.dram_tensor("bq", [PAIRS, 128], fp32, kind="ExternalInput").ap(),
        "bk": nc.dram_tensor("bk", [PAIRS, 128], fp32, kind="ExternalInput").ap(),
        "bv": nc.dram_tensor("bv", [256], fp32, kind="ExternalInput").ap(),
        "out": nc.dram_tensor("out", [PAIRS, 128, T], fp32,
                              kind="ExternalOutput").ap(),
    }
    with tile.TileContext(nc) as tc:
        _emit(tc, io)
    nc.compile()
    _CACHE["nc"] = nc
    return nc


def make_in_maps(inputs_x, k_w, k_b, q_w, q_b, v_w, v_b):
    """Host-side sharding: per-core input dict."""
    xs = [np.ascontiguousarray(inputs_x[b].T.astype(np.float16))
          for b in range(B)]
    in_maps = []
    for core in range(NCORES):
        b, g = divmod(core, 4)
        hs = range(4 * g, 4 * g + 4)
        pack_w = lambda w: np.ascontiguousarray(
            np.concatenate([w[h] for h in hs], axis=1).astype(np.float16))
        pack_b2 = lambda bb: np.ascontiguousarray(
            bb[4 * g:4 * g + 4].reshape(PAIRS, 128).astype(np.float32, copy=False))
        in_maps.append({
            "x_t": xs[b],
            "wq": pack_w(q_w), "wk": pack_w(k_w), "wv": pack_w(v_w),
            "bq": pack_b2(q_b), "bk": pack_b2(k_b),
            "bv": np.ascontiguousarray(
                v_b[4 * g:4 * g + 4].reshape(256).astype(np.float32, copy=False)),
        })
    return in_maps


def assemble(core_outs):
    """Gather per-core (PAIRS, 128, T) outputs into the full (B, T, H*D)."""
    out_full = np.empty((B, T, H * D), np.float32)
    for core in range(NCORES):
        b, g = divmod(core, 4)
        o = core_outs[core]
        for p in range(PAIRS):
            out_full[b, :, g * 256 + 128 * p:g * 256 + 128 * (p + 1)] = o[p].T
    return out_full


def kernel(**inputs):
    x = np.asarray(inputs["inputs"], np.float32)
    args = [np.asarray(inputs[k], np.float32)
            for k in ("k_w", "k_b", "q_w", "q_b", "v_w", "v_b")]
    from concourse.bass_utils import run_bass_kernel_spmd

    nc = _build()
    in_maps = make_in_maps(x, *args)
    res = run_bass_kernel_spmd(nc, in_maps, core_ids=list(range(NCORES)))
    return assemble([r["out"] for r in res.results])
